# revision 8
# baseline (speedup 1.0000x reference)
"""Trainium2 Bass kernel for a fused MultiHeadAttention block.

Reference computation (B=4, S=1024, D=1024, H=16, DK=DV=64):
    qh = einsum('bqd,hdk->bhqk', q, wq); kh, vh likewise
    attn = softmax(mask_fill(qh/sqrt(DK) @ kh^T))
    out  = LayerNorm(concat_heads(attn @ vh) @ fc_w.T + q) * ln_g + ln_b

Sharding: 8 shards = (batch b, seq half).  Each core owns 512 query rows of
one batch; K/V projections for that batch are computed redundantly by the
core pair.  Zero collectives.

v3 strategy (empirically driven, see HW microbenchmarks):
  - all projection GEMMs (q/k/v) and the fc GEMM run in fp8e4m3 with
    perf_mode=DoubleRow: one matmul contracts 256 rows (2 chunks), halving
    both instruction count and streamed columns. Host scales weights x16;
    descale factors fold into the exp scale and the fc evacuation.
  - scores run in bf16 with K=128: qh is stored zero-padded per head
    (qhz[:, hl] has the other head's 64 rows zeroed), because K=64 matmuls
    measure ~3x slower than K=128 on this hardware.
  - exp outputs fp8 directly with fused scale (1/2048 descale+temperature)
    and bias (-4 shift so exp(s-4) fits fp8e4's 240 max); mask multiply is
    fp8*fp8 split across Pool and DVE. PV is then fp8 DoubleRow with the
    p tiles already in [k,2,q] layout. Row sums come from a 4.0-column
    appended to vh; softmax needs no max pass.
  - psum evacuations: vh on Act (idle during the vh phase), kh/qh/fc on
    DVE; Pool handles all SBUF-side elementwise (mask, broadcast, LN
    affine) since GpSimd cannot touch PSUM.
  - inputs double-buffered (bufs=2) so rep i+1's DMAs prefetch during
    rep i; DMAs spread over sync/scalar/gpsimd queues in consumption
    order; output DMA split across queues.
"""

import os
import sys

import numpy as np

for _p in ("/opt/trn_rl_repo",):
    if _p not in sys.path and os.path.isdir(_p):
        sys.path.insert(0, _p)

from contextlib import ExitStack

import ml_dtypes

import concourse.bass as bass
import concourse.tile as tile
from concourse import bacc, mybir
from concourse.bass_utils import run_bass_kernel_spmd

F32 = mybir.dt.float32
BF16 = mybir.dt.bfloat16
FP8 = mybir.dt.float8e4
AF = mybir.ActivationFunctionType
DR = mybir.MatmulPerfMode.DoubleRow
NPBF16 = ml_dtypes.bfloat16
NPFP8 = ml_dtypes.float8_e4m3

B, S, D = 4, 1024, 1024
H, DK, DV = 16, 64, 64
SQ = S // 2          # query rows per core
P = 128
NDC = D // P         # 8 contraction chunks over D
ND2 = NDC // 2       # 4 DoubleRow chunks (256 rows each)
NKC = S // P         # 8 key chunks
NK2 = NKC // 2       # 4 DoubleRow key chunks
NQT = SQ // P        # 4 query subtiles
NPAIR = H // 2       # 8 head pairs
LN_EPS = 1e-6
N_CORES = 8
VW = DV + 1          # vh columns incl. the rowsum column
VPAD = 65            # vh stride (65 fp8 bytes -> H*VPAD = 1040 % 16 == 0)
NKCH = NKC // 2      # vh is split in two tiles of 4 key-chunks each
WSCALE = 16.0        # host-side weight scale for fp8
EXP_SCALE = 1.0 / (WSCALE * WSCALE * 8.0)   # descale qh*kh and temperature
EXP_SHIFT = -4.0     # keeps exp(s-4) under fp8e4's max of 240
SUM_COL = 4.0        # value of the vh rowsum column
FC_DESCALE = 1.0 / (WSCALE * SUM_COL)       # concat is 4*head, fc_w is x16


def build_program(reps: int = 1):
    nc = bacc.Bacc("TRN2", target_bir_lowering=False, debug=False)

    qT_d = nc.dram_tensor("qT_sh", [P, NDC, SQ], FP8, kind="ExternalInput")
    kT_d = nc.dram_tensor("kT_full", [P, NDC, S], FP8, kind="ExternalInput")
    vT_d = nc.dram_tensor("vT_full", [P, NDC, S], FP8, kind="ExternalInput")
    mT_d = nc.dram_tensor("mT_sh", [P, NKC, SQ], FP8, kind="ExternalInput")
    wq_d = nc.dram_tensor("wq_p", [P, NDC, H * DK], FP8, kind="ExternalInput")
    wk_d = nc.dram_tensor("wk_p", [P, NDC, H * DK], FP8, kind="ExternalInput")
    wv_d = nc.dram_tensor("wv_p", [P, NDC, H * DV], FP8, kind="ExternalInput")
    fcT_d = nc.dram_tensor("fcT_p", [P, NDC, D], FP8, kind="ExternalInput")
    qr_d = nc.dram_tensor("qr_sh", [P, NQT, D], BF16, kind="ExternalInput")
    g_d = nc.dram_tensor("ln_g", [D], BF16, kind="ExternalInput")
    b_d = nc.dram_tensor("ln_b", [D], BF16, kind="ExternalInput")
    o_d = nc.dram_tensor("out_sh", [SQ, D], F32, kind="ExternalOutput")

    with tile.TileContext(nc) as tc, ExitStack() as ctx:
        singles = ctx.enter_context(tc.tile_pool(name="singles", bufs=1))
        ins = ctx.enter_context(tc.tile_pool(name="ins", bufs=2))
        mid = ctx.enter_context(tc.tile_pool(name="mid", bufs=1))
        vha_pool = ctx.enter_context(tc.tile_pool(name="vha", bufs=2))
        vhb_pool = ctx.enter_context(tc.tile_pool(name="vhb", bufs=2))
        kh_pool = ctx.enter_context(tc.tile_pool(name="khp", bufs=2))
        qh_pool = ctx.enter_context(tc.tile_pool(name="qhp", bufs=2))
        pwork = ctx.enter_context(tc.tile_pool(name="pwork", bufs=4))
        work = ctx.enter_context(tc.tile_pool(name="work", bufs=2))
        ps_proj = ctx.enter_context(
            tc.tile_pool(name="ps_proj", bufs=2, space="PSUM"))
        ps_sc = ctx.enter_context(
            tc.tile_pool(name="ps_sc", bufs=2, space="PSUM"))
        ps_hd = ctx.enter_context(
            tc.tile_pool(name="ps_hd", bufs=2, space="PSUM"))

        eps1 = singles.tile([P, 1], F32, tag="eps1")
        nc.vector.memset(eps1, LN_EPS)
        bneg = singles.tile([P, 1], F32, tag="bneg")
        nc.vector.memset(bneg, EXP_SHIFT)

        def _one_rep():
            # -- input DMAs over three queues, in consumption order --
            wv_sb = ins.tile([P, NDC, H * DV], FP8, tag="wv")
            vT_sb = ins.tile([P, NDC, S], FP8, tag="vT")
            wk_sb = ins.tile([P, NDC, H * DK], FP8, tag="wk")
            wq_sb = ins.tile([P, NDC, H * DK], FP8, tag="wq")
            kT_sb = ins.tile([P, NDC, S], FP8, tag="kT")
            qT_sb = ins.tile([P, NDC, SQ], FP8, tag="qT")
            mT_sb = ins.tile([P, NKC, SQ], FP8, tag="mT")
            fcT_sb = ins.tile([P, NDC, D], FP8, tag="fcT")
            qr_sb = ins.tile([P, NQT, D], BF16, tag="qr")
            gb = ins.tile([P, 2, D], BF16, tag="gb")

            # all input DMAs on the SP queue: SP has no other work, so
            # rep i+1's issues execute during rep i (prefetch via bufs=2)
            nc.sync.dma_start(out=wv_sb, in_=wv_d[:])
            nc.sync.dma_start(out=vT_sb, in_=vT_d[:])
            nc.sync.dma_start(out=wk_sb, in_=wk_d[:])
            nc.sync.dma_start(out=wq_sb, in_=wq_d[:])
            nc.sync.dma_start(out=kT_sb, in_=kT_d[:])
            nc.sync.dma_start(out=qT_sb, in_=qT_d[:])
            nc.sync.dma_start(out=mT_sb, in_=mT_d[:])
            nc.sync.dma_start(out=fcT_sb, in_=fcT_d[:])
            nc.sync.dma_start(out=qr_sb, in_=qr_d[:])
            nc.sync.dma_start(
                out=gb[:, 0, :], in_=g_d.ap().unsqueeze(0).to_broadcast([P, D]))
            nc.sync.dma_start(
                out=gb[:, 1, :], in_=b_d.ap().unsqueeze(0).to_broadcast([P, D]))

            # -- vh projection (fp8 DoubleRow): vh[key_p, kc, h, 0:64],
            #    col 64 = SUM_COL for softmax row sums --
            vhA = vha_pool.tile([P, NKCH, H, VPAD], FP8, tag="vhA")
            vhB = vhb_pool.tile([P, NKCH, H, VPAD], FP8, tag="vhB")

            def vh_tile(kc):
                t = vhA if kc < NKCH else vhB
                return t[:, kc % NKCH]

            nc.gpsimd.memset(vhA[:, :, :, DV:DV + 1], SUM_COL)
            nc.gpsimd.memset(vhB[:, :, :, DV:DV + 1], SUM_COL)
            for kc in range(NKC):
                for hf in range(2):
                    vps = ps_proj.tile([P, 512], F32, tag="proj")
                    for d2 in range(ND2):
                        nc.tensor.matmul(
                            vps,
                            lhsT=vT_sb[:, 2 * d2:2 * d2 + 2,
                                       kc * P:(kc + 1) * P],
                            rhs=wv_sb[:, 2 * d2:2 * d2 + 2,
                                      hf * 512:(hf + 1) * 512],
                            start=(d2 == 0), stop=(d2 == ND2 - 1),
                            perf_mode=DR)
                    nc.scalar.copy(
                        out=vh_tile(kc)[:, hf * 8:(hf + 1) * 8, 0:DV],
                        in_=vps.rearrange("p (h v) -> p h v", v=DV))

            # -- per head-pair attention with interleaved projections.
            # proj matmuls of pair p+1 fill pair p's PE gaps; each PV
            # trails its scores by 2 groups so PE never waits on exp. --
            concatT = mid.tile([P, NPAIR, SQ], FP8, tag="concatT")

            def proj_gen(pair):
                """Yields after each of the 12 DR proj matmuls of `pair`."""
                cols = slice(pair * P, (pair + 1) * P)
                kh_t = kh_pool.tile([P, S], BF16, tag="kh", name="kh_t")
                qh_t = qh_pool.tile([P, 2, SQ], BF16, tag="qhz", name="qh_t")
                PAIR_TILES.append((kh_t, qh_t))
                # zero the pad halves: qhz[:,0] keeps head0 rows 0:64,
                # qhz[:,1] keeps head1 rows 64:128
                nc.gpsimd.memset(qh_t[DK:P, 0, :], 0.0)
                nc.gpsimd.memset(qh_t[0:DK, 1, :], 0.0)
                qhps = ps_proj.tile([P, 512], F32, tag="proj")
                for d2 in range(ND2):
                    nc.tensor.matmul(
                        qhps,
                        lhsT=wq_sb[:, 2 * d2:2 * d2 + 2, cols],
                        rhs=qT_sb[:, 2 * d2:2 * d2 + 2, :],
                        start=(d2 == 0), stop=(d2 == ND2 - 1), perf_mode=DR)
                    if d2 == ND2 - 1:
                        nc.vector.tensor_copy(
                            out=qh_t[0:DK, 0, :], in_=qhps[0:DK, :])
                        nc.vector.tensor_copy(
                            out=qh_t[DK:P, 1, :], in_=qhps[DK:P, :])
                    yield
                for hf in range(2):
                    khps = ps_proj.tile([P, 512], F32, tag="proj")
                    for d2 in range(ND2):
                        nc.tensor.matmul(
                            khps,
                            lhsT=wk_sb[:, 2 * d2:2 * d2 + 2, cols],
                            rhs=kT_sb[:, 2 * d2:2 * d2 + 2,
                                      hf * 512:(hf + 1) * 512],
                            start=(d2 == 0), stop=(d2 == ND2 - 1),
                            perf_mode=DR)
                        if d2 == ND2 - 1:
                            nc.vector.tensor_copy(
                                out=kh_t[:, hf * 512:(hf + 1) * 512],
                                in_=khps)
                        yield

            PAIR_TILES = []

            def drain(gen, n):
                if gen is None:
                    return
                for _ in range(n):
                    next(gen, None)

            hd_map = {}

            def emit_pv(ent):
                h, kc2, p_t = ent
                if h not in hd_map:
                    hd_map[h] = ps_hd.tile([P, SQ], F32, tag="hd", name="hd")
                hd = hd_map[h]
                ko = (2 * kc2) % 4
                vt = vhA if kc2 < NK2 // 2 else vhB
                nc.tensor.matmul(
                    hd[0:VW, :],
                    lhsT=vt[:, ko:ko + 2, h, 0:VW],
                    rhs=p_t,
                    start=(kc2 == 0), stop=(kc2 == NK2 - 1), perf_mode=DR)
                if kc2 == NK2 - 1:
                    # normalize rows 0:64 by the rowsum in row 64
                    hl = h % 2
                    recip = work.tile([1, SQ], F32, tag="recip")
                    nc.vector.reciprocal(out=recip, in_=hd[DV:DV + 1, :])
                    recip_bc = work.tile([DV, SQ], F32, tag="recip_bc")
                    nc.gpsimd.partition_broadcast(recip_bc, recip)
                    nc.vector.tensor_mul(
                        concatT[hl * DV:(hl + 1) * DV, h // 2, :],
                        hd[0:DV, :], recip_bc)
                    del hd_map[h]

            drain(proj_gen(0), 12)
            kh_cur, qh_cur = PAIR_TILES.pop()
            pending = []
            for pair in range(NPAIR):
                filler = proj_gen(pair + 1) if pair + 1 < NPAIR else None
                fill_sched = (2, 1, 2, 1, 2, 1, 2, 1)
                g = 0
                for hl in range(2):
                    h = 2 * pair + hl
                    for kc2 in range(NK2):
                        sc = ps_sc.tile([P, 2, SQ], F32, tag="sc")
                        for j in range(2):
                            kc = 2 * kc2 + j
                            nc.tensor.matmul(
                                sc[:, j, :],
                                lhsT=kh_cur[:, kc * P:(kc + 1) * P],
                                rhs=qh_cur[:, hl, :],
                                start=True, stop=True)
                        p_t = pwork.tile([P, 2, SQ], FP8, tag="p_t")
                        nc.scalar.activation(
                            out=p_t, in_=sc, func=AF.Exp,
                            scale=EXP_SCALE, bias=bneg)
                        meng = nc.gpsimd if kc2 < NK2 - 1 else nc.vector
                        meng.tensor_mul(
                            p_t, p_t, mT_sb[:, 2 * kc2:2 * kc2 + 2, :])
                        drain(filler, fill_sched[g])
                        g += 1
                        pending.append((h, kc2, p_t))
                        if len(pending) > 2:
                            emit_pv(pending.pop(0))
                if filler is not None:
                    drain(filler, 12)  # safety: ensure fully drained
                    kh_cur, qh_cur = PAIR_TILES.pop()
            for ent in pending:
                emit_pv(ent)

            # -- fc (fp8 DoubleRow) + residual + LayerNorm, two waves of
            #    (st, hf) groups to fit psum --
            for st in range(NQT):
                o_sb = work.tile([P, D], F32, tag="o_sb")
                for hf in range(2):
                    fps = ps_proj.tile([P, 512], F32, tag="proj")
                    for i2 in range(ND2):
                        nc.tensor.matmul(
                            fps,
                            lhsT=concatT[:, 2 * i2:2 * i2 + 2,
                                         st * P:(st + 1) * P],
                            rhs=fcT_sb[:, 2 * i2:2 * i2 + 2,
                                       hf * 512:(hf + 1) * 512],
                            start=(i2 == 0), stop=(i2 == ND2 - 1),
                            perf_mode=DR)
                    nc.vector.scalar_tensor_tensor(
                        out=o_sb[:, hf * 512:(hf + 1) * 512],
                        in0=fps, scalar=FC_DESCALE,
                        in1=qr_sb[:, st, hf * 512:(hf + 1) * 512],
                        op0=mybir.AluOpType.mult, op1=mybir.AluOpType.add)
                stats = work.tile([P, 2, 6], F32, tag="stats")
                for sg in range(2):
                    nc.vector.bn_stats(
                        out=stats[:, sg, :],
                        in_=o_sb[:, sg * 512:(sg + 1) * 512])
                mv = work.tile([P, 2], F32, tag="mv")
                nc.vector.bn_aggr(out=mv, in_=stats)
                std = work.tile([P, 1], F32, tag="std")
                nc.scalar.activation(
                    out=std, in_=mv[:, 1:2], func=AF.Sqrt, bias=eps1)
                rstd = work.tile([P, 1], F32, tag="rstd")
                nc.vector.reciprocal(out=rstd, in_=std)
                nc.vector.tensor_scalar(
                    out=o_sb, in0=o_sb, scalar1=mv[:, 0:1], scalar2=rstd,
                    op0=mybir.AluOpType.subtract, op1=mybir.AluOpType.mult)
                nc.gpsimd.tensor_mul(o_sb, o_sb, gb[:, 0, :])
                nc.gpsimd.tensor_add(o_sb, o_sb, gb[:, 1, :])
                oq = nc.scalar if st % 2 == 0 else nc.gpsimd
                oq.dma_start(out=o_d[st * P:(st + 1) * P, :], in_=o_sb)

        for _rep in range(reps):
            _one_rep()

    nc.compile()
    return nc


_CACHE = {}


def _get_program():
    if "nc" not in _CACHE:
        _CACHE["nc"] = build_program()
    return _CACHE["nc"]


def _to_pds(x_t, nfree, dtype):
    """[d, n] (d-major) -> [128, d//128, n] partition-dim-split layout."""
    d = x_t.shape[0]
    return np.ascontiguousarray(
        x_t.reshape(d // P, P, nfree).transpose(1, 0, 2).astype(dtype))


def make_in_maps(q, k, v, mask, wq, wk, wv, fc_w, ln_g, ln_b):
    q = np.asarray(q, dtype=np.float32)
    k = np.asarray(k, dtype=np.float32)
    v = np.asarray(v, dtype=np.float32)
    mask = np.asarray(mask, dtype=np.int32)
    # weights, packed [p, dc, h*64+j], scaled x16, fp8 (shared by all cores)
    wq_p = _to_pds(np.asarray(wq).transpose(1, 0, 2).reshape(D, H * DK)
                   * WSCALE, H * DK, NPFP8)
    wk_p = _to_pds(np.asarray(wk).transpose(1, 0, 2).reshape(D, H * DK)
                   * WSCALE, H * DK, NPFP8)
    wv_p = _to_pds(np.asarray(wv).transpose(1, 0, 2).reshape(D, H * DV)
                   * WSCALE, H * DV, NPFP8)
    fcT_p = _to_pds(np.asarray(fc_w, dtype=np.float32).T * WSCALE, D, NPFP8)
    shared = {
        "wq_p": wq_p, "wk_p": wk_p, "wv_p": wv_p, "fcT_p": fcT_p,
        "ln_g": np.ascontiguousarray(np.asarray(ln_g).astype(NPBF16)),
        "ln_b": np.ascontiguousarray(np.asarray(ln_b).astype(NPBF16)),
    }
    in_maps = []
    for c in range(N_CORES):
        b, half = c // 2, c % 2
        sl = slice(half * SQ, (half + 1) * SQ)
        q_sl = q[b, sl, :]
        in_maps.append({
            "qT_sh": _to_pds(q_sl.T, SQ, NPFP8),
            "kT_full": _to_pds(k[b].T, S, NPFP8),
            "vT_full": _to_pds(v[b].T, S, NPFP8),
            "mT_sh": _to_pds(mask[b, sl, :].T, SQ, NPFP8),
            "qr_sh": np.ascontiguousarray(
                q_sl.astype(NPBF16).reshape(NQT, P, D).transpose(1, 0, 2)),
            **shared,
        })
    return in_maps


def run(inputs: dict, trace: bool = False):
    nc = _get_program()
    in_maps = make_in_maps(**inputs)
    res = run_bass_kernel_spmd(
        nc, in_maps, core_ids=list(range(N_CORES)), trace=trace)
    out = np.empty((B, S, D), dtype=np.float32)
    for c in range(N_CORES):
        b, half = c // 2, c % 2
        out[b, half * SQ:(half + 1) * SQ, :] = res.results[c]["out_sh"]
    return out, res


def kernel(q, k, v, mask, wq, wk, wv, fc_w, ln_g, ln_b):
    out, _ = run(dict(q=q, k=k, v=v, mask=mask, wq=wq, wk=wk, wv=wv,
                      fc_w=fc_w, ln_g=ln_g, ln_b=ln_b))
    return out


# revision 10
# speedup vs baseline: 2.1002x; 2.1002x over previous
"""Trainium2 Bass kernel for a fused MultiHeadAttention block.

Reference computation (B=4, S=1024, D=1024, H=16, DK=DV=64):
    qh = einsum('bqd,hdk->bhqk', q, wq); kh, vh likewise
    attn = softmax(mask_fill(qh/sqrt(DK) @ kh^T))
    out  = LayerNorm(concat_heads(attn @ vh) @ fc_w.T + q) * ln_g + ln_b

Sharding: 8 shards = (batch b, seq half).  Each core owns 512 query rows of
one batch; K/V projections for that batch are computed redundantly by the
core pair.  Zero collectives.

v3 strategy (empirically driven, see HW microbenchmarks):
  - all projection GEMMs (q/k/v) and the fc GEMM run in fp8e4m3 with
    perf_mode=DoubleRow: one matmul contracts 256 rows (2 chunks), halving
    both instruction count and streamed columns. Host scales weights x16;
    descale factors fold into the exp scale and the fc evacuation.
  - scores run in bf16 with K=128: qh is stored zero-padded per head
    (qhz[:, hl] has the other head's 64 rows zeroed), because K=64 matmuls
    measure ~3x slower than K=128 on this hardware.
  - exp outputs fp8 directly with fused scale (1/2048 descale+temperature)
    and bias (-4 shift so exp(s-4) fits fp8e4's 240 max); mask multiply is
    fp8*fp8 split across Pool and DVE. PV is then fp8 DoubleRow with the
    p tiles already in [k,2,q] layout. Row sums come from a 4.0-column
    appended to vh; softmax needs no max pass.
  - psum evacuations: vh on Act (idle during the vh phase), kh/qh/fc on
    DVE; Pool handles all SBUF-side elementwise (mask, broadcast, LN
    affine) since GpSimd cannot touch PSUM.
  - inputs double-buffered (bufs=2) so rep i+1's DMAs prefetch during
    rep i; DMAs spread over sync/scalar/gpsimd queues in consumption
    order; output DMA split across queues.
"""

import os
import sys

import numpy as np

for _p in ("/opt/trn_rl_repo",):
    if _p not in sys.path and os.path.isdir(_p):
        sys.path.insert(0, _p)

from contextlib import ExitStack

import ml_dtypes

import concourse.bass as bass
import concourse.tile as tile
from concourse import bacc, mybir
from concourse.bass_utils import run_bass_kernel_spmd

F32 = mybir.dt.float32
BF16 = mybir.dt.bfloat16
FP8 = mybir.dt.float8e4
AF = mybir.ActivationFunctionType
DR = mybir.MatmulPerfMode.DoubleRow
NPBF16 = ml_dtypes.bfloat16
NPFP8 = ml_dtypes.float8_e4m3

B, S, D = 4, 1024, 1024
H, DK, DV = 16, 64, 64
SQ = S // 2          # query rows per core
P = 128
NDC = D // P         # 8 contraction chunks over D
ND2 = NDC // 2       # 4 DoubleRow chunks (256 rows each)
NKC = S // P         # 8 key chunks
NK2 = NKC // 2       # 4 DoubleRow key chunks
NQT = SQ // P        # 4 query subtiles
NPAIR = H // 2       # 8 head pairs
LN_EPS = 1e-6
N_CORES = 8
VW = DV + 1          # vh columns incl. the rowsum column
VPAD = 65            # vh stride (65 fp8 bytes -> H*VPAD = 1040 % 16 == 0)
NKCH = NKC // 2      # vh is split in two tiles of 4 key-chunks each
WSCALE = 16.0        # host-side weight scale for fp8
EXP_SCALE = 1.0 / (WSCALE * WSCALE * 8.0)   # descale qh*kh and temperature
EXP_SHIFT = -4.0     # keeps exp(s-4) under fp8e4's max of 240
SUM_COL = 4.0        # value of the vh rowsum column
FC_DESCALE = 1.0 / (WSCALE * SUM_COL)       # concat is 4*head, fc_w is x16


def build_program(reps: int = 1):
    nc = bacc.Bacc("TRN2", target_bir_lowering=False, debug=False)

    qT_d = nc.dram_tensor("qT_sh", [P, NDC, SQ], FP8, kind="ExternalInput")
    kT_d = nc.dram_tensor("kT_full", [P, NDC, S], FP8, kind="ExternalInput")
    vT_d = nc.dram_tensor("vT_full", [P, NDC, S], FP8, kind="ExternalInput")
    mT_d = nc.dram_tensor("mT_sh", [P, NKC, SQ], BF16, kind="ExternalInput")
    wq_d = nc.dram_tensor("wq_p", [P, NDC, H * DK], FP8, kind="ExternalInput")
    wk_d = nc.dram_tensor("wk_p", [P, NDC, H * DK], FP8, kind="ExternalInput")
    wv_d = nc.dram_tensor("wv_p", [P, NDC, H * DV], FP8, kind="ExternalInput")
    fcT_d = nc.dram_tensor("fcT_p", [P, NDC, D], FP8, kind="ExternalInput")
    qr_d = nc.dram_tensor("qr_sh", [P, NQT, D], BF16, kind="ExternalInput")
    g_d = nc.dram_tensor("ln_g", [D], BF16, kind="ExternalInput")
    b_d = nc.dram_tensor("ln_b", [D], BF16, kind="ExternalInput")
    o_d = nc.dram_tensor("out_sh", [SQ, D], F32, kind="ExternalOutput")

    with tile.TileContext(nc) as tc, ExitStack() as ctx:
        singles = ctx.enter_context(tc.tile_pool(name="singles", bufs=1))
        ins = ctx.enter_context(tc.tile_pool(name="ins", bufs=2))
        ins1 = ctx.enter_context(tc.tile_pool(name="ins1", bufs=1))
        mid = ctx.enter_context(tc.tile_pool(name="mid", bufs=1))
        vha_pool = ctx.enter_context(tc.tile_pool(name="vha", bufs=2))
        vhb_pool = ctx.enter_context(tc.tile_pool(name="vhb", bufs=2))
        kh_pool = ctx.enter_context(tc.tile_pool(name="khp", bufs=2))
        qh_pool = ctx.enter_context(tc.tile_pool(name="qhp", bufs=2))
        pwork = ctx.enter_context(tc.tile_pool(name="pwork", bufs=4))
        work = ctx.enter_context(tc.tile_pool(name="work", bufs=2))
        ps_proj = ctx.enter_context(
            tc.tile_pool(name="ps_proj", bufs=2, space="PSUM"))
        ps_sc = ctx.enter_context(
            tc.tile_pool(name="ps_sc", bufs=2, space="PSUM"))
        ps_hd = ctx.enter_context(
            tc.tile_pool(name="ps_hd", bufs=2, space="PSUM"))

        eps1 = singles.tile([P, 1], F32, tag="eps1")
        nc.vector.memset(eps1, LN_EPS)
        bneg = singles.tile([P, 1], F32, tag="bneg")
        nc.vector.memset(bneg, EXP_SHIFT)

        def _one_rep():
            # -- input DMAs over three queues, in consumption order --
            wv_sb = ins.tile([P, NDC, H * DV], FP8, tag="wv")
            vT_sb = ins.tile([P, NDC, S], FP8, tag="vT")
            wk_sb = ins.tile([P, NDC, H * DK], FP8, tag="wk")
            wq_sb = ins.tile([P, NDC, H * DK], FP8, tag="wq")
            kT_sb = ins.tile([P, NDC, S], FP8, tag="kT")
            qT_sb = ins.tile([P, NDC, SQ], FP8, tag="qT")
            mT_sb = ins.tile([P, NKC, SQ], BF16, tag="mT")
            fcT_sb = ins1.tile([P, NDC, D], FP8, tag="fcT")
            qr_sb = ins1.tile([P, NQT, D], BF16, tag="qr")
            gb = ins1.tile([P, 2, D], BF16, tag="gb")

            # all input DMAs on the SP queue: SP has no other work, so
            # rep i+1's issues execute during rep i (prefetch via bufs=2)
            nc.sync.dma_start(out=wv_sb, in_=wv_d[:])
            nc.sync.dma_start(out=vT_sb, in_=vT_d[:])
            nc.sync.dma_start(out=wk_sb, in_=wk_d[:])
            nc.sync.dma_start(out=wq_sb, in_=wq_d[:])
            nc.sync.dma_start(out=kT_sb, in_=kT_d[:])
            nc.sync.dma_start(out=qT_sb, in_=qT_d[:])
            nc.sync.dma_start(out=mT_sb, in_=mT_d[:])
            nc.sync.dma_start(out=fcT_sb, in_=fcT_d[:])
            nc.sync.dma_start(out=qr_sb, in_=qr_d[:])
            nc.sync.dma_start(
                out=gb[:, 0, :], in_=g_d.ap().unsqueeze(0).to_broadcast([P, D]))
            nc.sync.dma_start(
                out=gb[:, 1, :], in_=b_d.ap().unsqueeze(0).to_broadcast([P, D]))

            # -- vh projection (fp8 DoubleRow): vh[key_p, kc, h, 0:64],
            #    col 64 = SUM_COL for softmax row sums --
            vhA = vha_pool.tile([P, NKCH, H, VPAD], BF16, tag="vhA")
            vhB = vhb_pool.tile([P, NKCH, H, VPAD], BF16, tag="vhB")

            def vh_tile(kc):
                t = vhA if kc < NKCH else vhB
                return t[:, kc % NKCH]

            nc.gpsimd.memset(vhA[:, :, :, DV:DV + 1], SUM_COL)
            nc.gpsimd.memset(vhB[:, :, :, DV:DV + 1], SUM_COL)
            for kc in range(NKC):
                for hf in range(2):
                    vps = ps_proj.tile([P, 512], F32, tag="proj")
                    for d2 in range(ND2):
                        nc.tensor.matmul(
                            vps,
                            lhsT=vT_sb[:, 2 * d2:2 * d2 + 2,
                                       kc * P:(kc + 1) * P],
                            rhs=wv_sb[:, 2 * d2:2 * d2 + 2,
                                      hf * 512:(hf + 1) * 512],
                            start=(d2 == 0), stop=(d2 == ND2 - 1),
                            perf_mode=DR)
                    nc.scalar.copy(
                        out=vh_tile(kc)[:, hf * 8:(hf + 1) * 8, 0:DV],
                        in_=vps.rearrange("p (h v) -> p h v", v=DV))

            # -- per head-pair attention with interleaved projections.
            # proj matmuls of pair p+1 fill pair p's PE gaps; each PV
            # trails its scores by 2 groups so PE never waits on exp. --
            concatT = mid.tile([P, NPAIR, SQ], FP8, tag="concatT")

            def proj_gen(pair):
                """Yields after each of the 12 DR proj matmuls of `pair`."""
                cols = slice(pair * P, (pair + 1) * P)
                kh_t = kh_pool.tile([P, S], BF16, tag="kh", name="kh_t")
                qh_t = qh_pool.tile([P, 2, SQ], BF16, tag="qhz", name="qh_t")
                PAIR_TILES.append((kh_t, qh_t))
                # zero the pad halves: qhz[:,0] keeps head0 rows 0:64,
                # qhz[:,1] keeps head1 rows 64:128
                nc.gpsimd.memset(qh_t[DK:P, 0, :], 0.0)
                nc.gpsimd.memset(qh_t[0:DK, 1, :], 0.0)
                qhps = ps_proj.tile([P, 512], F32, tag="proj")
                for d2 in range(ND2):
                    nc.tensor.matmul(
                        qhps,
                        lhsT=wq_sb[:, 2 * d2:2 * d2 + 2, cols],
                        rhs=qT_sb[:, 2 * d2:2 * d2 + 2, :],
                        start=(d2 == 0), stop=(d2 == ND2 - 1), perf_mode=DR)
                    if d2 == ND2 - 1:
                        nc.vector.tensor_copy(
                            out=qh_t[0:DK, 0, :], in_=qhps[0:DK, :])
                        nc.vector.tensor_copy(
                            out=qh_t[DK:P, 1, :], in_=qhps[DK:P, :])
                    yield
                for hf in range(2):
                    khps = ps_proj.tile([P, 512], F32, tag="proj")
                    for d2 in range(ND2):
                        nc.tensor.matmul(
                            khps,
                            lhsT=wk_sb[:, 2 * d2:2 * d2 + 2, cols],
                            rhs=kT_sb[:, 2 * d2:2 * d2 + 2,
                                      hf * 512:(hf + 1) * 512],
                            start=(d2 == 0), stop=(d2 == ND2 - 1),
                            perf_mode=DR)
                        if d2 == ND2 - 1:
                            nc.vector.tensor_copy(
                                out=kh_t[:, hf * 512:(hf + 1) * 512],
                                in_=khps)
                        yield

            PAIR_TILES = []

            def drain(gen, n):
                if gen is None:
                    return
                for _ in range(n):
                    next(gen, None)

            hd_map = {}

            def emit_pv(ent):
                h, kc2, p_t = ent
                if h not in hd_map:
                    hd_map[h] = ps_hd.tile([P, SQ], F32, tag="hd", name="hd")
                hd = hd_map[h]
                for j in range(2):
                    kc = 2 * kc2 + j
                    nc.tensor.matmul(
                        hd[0:VW, :],
                        lhsT=vh_tile(kc)[:, h, 0:VW],
                        rhs=p_t[:, j, :],
                        start=(kc == 0), stop=(kc == NKC - 1))
                if kc2 == NK2 - 1:
                    # normalize rows 0:64 by the rowsum in row 64
                    hl = h % 2
                    recip = work.tile([1, SQ], F32, tag="recip")
                    nc.vector.reciprocal(out=recip, in_=hd[DV:DV + 1, :])
                    recip_bc = work.tile([DV, SQ], F32, tag="recip_bc")
                    nc.gpsimd.partition_broadcast(recip_bc, recip)
                    nc.vector.tensor_mul(
                        concatT[hl * DV:(hl + 1) * DV, h // 2, :],
                        hd[0:DV, :], recip_bc)
                    del hd_map[h]

            drain(proj_gen(0), 12)
            kh_cur, qh_cur = PAIR_TILES.pop()
            pending = []
            for pair in range(NPAIR):
                filler = proj_gen(pair + 1) if pair + 1 < NPAIR else None
                fill_sched = (2, 1, 2, 1, 2, 1, 2, 1)
                g = 0
                for hl in range(2):
                    h = 2 * pair + hl
                    for kc2 in range(NK2):
                        sc = ps_sc.tile([P, 2, SQ], F32, tag="sc")
                        for j in range(2):
                            kc = 2 * kc2 + j
                            nc.tensor.matmul(
                                sc[:, j, :],
                                lhsT=kh_cur[:, kc * P:(kc + 1) * P],
                                rhs=qh_cur[:, hl, :],
                                start=True, stop=True)
                        p_t = pwork.tile([P, 2, SQ], BF16, tag="p_t")
                        nc.scalar.activation(
                            out=p_t, in_=sc, func=AF.Exp,
                            scale=EXP_SCALE, bias=bneg)
                        nc.vector.tensor_mul(
                            p_t, p_t, mT_sb[:, 2 * kc2:2 * kc2 + 2, :])
                        drain(filler, fill_sched[g])
                        g += 1
                        pending.append((h, kc2, p_t))
                        if len(pending) > 2:
                            emit_pv(pending.pop(0))
                if filler is not None:
                    drain(filler, 12)  # safety: ensure fully drained
                    kh_cur, qh_cur = PAIR_TILES.pop()
            for ent in pending:
                emit_pv(ent)

            # -- fc (fp8 DoubleRow) + residual + LayerNorm, two waves of
            #    (st, hf) groups to fit psum --
            for st in range(NQT):
                o_sb = work.tile([P, D], F32, tag="o_sb")
                for hf in range(2):
                    fps = ps_proj.tile([P, 512], F32, tag="proj")
                    for i2 in range(ND2):
                        nc.tensor.matmul(
                            fps,
                            lhsT=concatT[:, 2 * i2:2 * i2 + 2,
                                         st * P:(st + 1) * P],
                            rhs=fcT_sb[:, 2 * i2:2 * i2 + 2,
                                       hf * 512:(hf + 1) * 512],
                            start=(i2 == 0), stop=(i2 == ND2 - 1),
                            perf_mode=DR)
                    nc.vector.scalar_tensor_tensor(
                        out=o_sb[:, hf * 512:(hf + 1) * 512],
                        in0=fps, scalar=FC_DESCALE,
                        in1=qr_sb[:, st, hf * 512:(hf + 1) * 512],
                        op0=mybir.AluOpType.mult, op1=mybir.AluOpType.add)
                stats = work.tile([P, 2, 6], F32, tag="stats")
                for sg in range(2):
                    nc.vector.bn_stats(
                        out=stats[:, sg, :],
                        in_=o_sb[:, sg * 512:(sg + 1) * 512])
                mv = work.tile([P, 2], F32, tag="mv")
                nc.vector.bn_aggr(out=mv, in_=stats)
                std = work.tile([P, 1], F32, tag="std")
                nc.scalar.activation(
                    out=std, in_=mv[:, 1:2], func=AF.Sqrt, bias=eps1)
                rstd = work.tile([P, 1], F32, tag="rstd")
                nc.vector.reciprocal(out=rstd, in_=std)
                nc.vector.tensor_scalar(
                    out=o_sb, in0=o_sb, scalar1=mv[:, 0:1], scalar2=rstd,
                    op0=mybir.AluOpType.subtract, op1=mybir.AluOpType.mult)
                nc.gpsimd.tensor_mul(o_sb, o_sb, gb[:, 0, :])
                nc.gpsimd.tensor_add(o_sb, o_sb, gb[:, 1, :])
                oq = nc.scalar if st % 2 == 0 else nc.gpsimd
                oq.dma_start(out=o_d[st * P:(st + 1) * P, :], in_=o_sb)

        for _rep in range(reps):
            _one_rep()

    nc.compile()
    return nc


_CACHE = {}


def _get_program():
    if "nc" not in _CACHE:
        _CACHE["nc"] = build_program()
    return _CACHE["nc"]


def _to_pds(x_t, nfree, dtype):
    """[d, n] (d-major) -> [128, d//128, n] partition-dim-split layout."""
    d = x_t.shape[0]
    return np.ascontiguousarray(
        x_t.reshape(d // P, P, nfree).transpose(1, 0, 2).astype(dtype))


def make_in_maps(q, k, v, mask, wq, wk, wv, fc_w, ln_g, ln_b):
    q = np.asarray(q, dtype=np.float32)
    k = np.asarray(k, dtype=np.float32)
    v = np.asarray(v, dtype=np.float32)
    mask = np.asarray(mask, dtype=np.int32)
    # weights, packed [p, dc, h*64+j], scaled x16, fp8 (shared by all cores)
    wq_p = _to_pds(np.asarray(wq).transpose(1, 0, 2).reshape(D, H * DK)
                   * WSCALE, H * DK, NPFP8)
    wk_p = _to_pds(np.asarray(wk).transpose(1, 0, 2).reshape(D, H * DK)
                   * WSCALE, H * DK, NPFP8)
    wv_p = _to_pds(np.asarray(wv).transpose(1, 0, 2).reshape(D, H * DV)
                   * WSCALE, H * DV, NPFP8)
    fcT_p = _to_pds(np.asarray(fc_w, dtype=np.float32).T * WSCALE, D, NPFP8)
    shared = {
        "wq_p": wq_p, "wk_p": wk_p, "wv_p": wv_p, "fcT_p": fcT_p,
        "ln_g": np.ascontiguousarray(np.asarray(ln_g).astype(NPBF16)),
        "ln_b": np.ascontiguousarray(np.asarray(ln_b).astype(NPBF16)),
    }
    in_maps = []
    for c in range(N_CORES):
        b, half = c // 2, c % 2
        sl = slice(half * SQ, (half + 1) * SQ)
        q_sl = q[b, sl, :]
        in_maps.append({
            "qT_sh": _to_pds(q_sl.T, SQ, NPFP8),
            "kT_full": _to_pds(k[b].T, S, NPFP8),
            "vT_full": _to_pds(v[b].T, S, NPFP8),
            "mT_sh": _to_pds(mask[b, sl, :].T, SQ, NPBF16),
            "qr_sh": np.ascontiguousarray(
                q_sl.astype(NPBF16).reshape(NQT, P, D).transpose(1, 0, 2)),
            **shared,
        })
    return in_maps


def run(inputs: dict, trace: bool = False):
    nc = _get_program()
    in_maps = make_in_maps(**inputs)
    res = run_bass_kernel_spmd(
        nc, in_maps, core_ids=list(range(N_CORES)), trace=trace)
    out = np.empty((B, S, D), dtype=np.float32)
    for c in range(N_CORES):
        b, half = c // 2, c % 2
        out[b, half * SQ:(half + 1) * SQ, :] = res.results[c]["out_sh"]
    return out, res


def kernel(q, k, v, mask, wq, wk, wv, fc_w, ln_g, ln_b):
    out, _ = run(dict(q=q, k=k, v=v, mask=mask, wq=wq, wk=wk, wv=wv,
                      fc_w=fc_w, ln_g=ln_g, ln_b=ln_b))
    return out
